# revision 75
# baseline (speedup 1.0000x reference)
"""Trainium2 Bass kernel for an involution Bottleneck block (B=2, Cin=256,
Cmid=64, Cout=256, H=W=56, K=15, G=4).

Sharding: 8 cores = 2 batches x 4 H-quarters (14 output rows each). Each core
receives a zero-padded input halo [256, 28, 70] (7 rows/cols each side), so no
inter-core communication is needed (halo compute is redundant).

Per-core pipeline (channels on SBUF partitions, pixels on free dim):
  conv1 1x1 (PE, bf16) -> BN+ReLU (ACT) -> out1 [64, 28x70] bf16, plus a copy
    shifted by +7 rows at partitions 64:128 and a +1-col copy (odd-kx align).
  involution as 113 tap-PAIRS (ky, ky+7): per pair one span matmul with M=128
    (2 taps x 64 group-expanded channels, bias via the ones-row at K=17) into
    an 8-bank PSUM ring.  Pairs are processed as same-parity-kx QUADS: ACT
    evicts the quad to bf16 SBUF, then ONE fused DVE tensor_mul covers both
    pairs.  ~6 "direct" quads skip the eviction: DVE multiplies straight from
    PSUM fp32 (1x DVE rate but zero ACT cost).  Accumulation runs as two
    incremental chains: DVE (fast) and GpSimd (slow but otherwise idle).
  merge on the PE (ident2 fold of the 2-tap partitions + all chain planes into
    PSUM) -> gamma2 scale + b2 + ReLU (ACT, per-partition scale) -> conv3 1x1
    with g3 folded into W3 (PE) accumulating the bf16 residual via an identity
    matmul -> +b3, ReLU (ACT) -> bf16 DMA out (host converts to f32).

DMA descriptor generation is ~25ns/row and serial per sequencer (~6us per
input-sized dma_start), so input loads and output stores are issued from
different engine sequencers to overlap their descriptor generation.
"""

import sys, types
sys.path.insert(0, "/opt/trn_rl_repo")

import numpy as np
import ml_dtypes
from contextlib import ExitStack

import concourse.bass as bass
import concourse.mybir as mybir
import concourse.tile as tile
from concourse import bacc
from concourse.bass import ts
from concourse.bass_utils import run_bass_kernel_spmd

BF16 = mybir.dt.bfloat16
FP8 = mybir.dt.float8e4
F32 = mybir.dt.float32
AF = mybir.ActivationFunctionType

K = 15
G = 4
GC = 16
PAD = 7
CIN = 256
CMID = 64
RED = 16
COUT = 256
H = 56
W = 56
B = 2
HB = 14            # output rows per core
HP = HB + 2 * PAD  # 28 padded rows
WP = W + 2 * PAD   # 70 padded cols
NP = HP * WP       # 1960
HH = HB // 2       # 7 rows per half-block
NF = HH * W        # 392 pixels per half-block
NPAIR = 7 * K + 8  # 105 (ky,ky+7) pairs + 8 row-14 (kx,kx+1) pairs = 113
WCH = 38           # wse pairs per 32-partition chunk

_PROGRAM = None  # (nc, names) cache


def _fused4d(ap_like, base, pair_stride, row_stride):
    """4D AP [128][2 pairs][14 rows][56 cols] at arbitrary strides."""
    s_ = ap_like[:, base:base + 1]
    return bass.AP(tensor=s_.tensor, offset=s_.offset,
                   ap=[list(s_.ap[0]), [pair_stride, 2], [row_stride, 14],
                       [1, 56]])


def _build_program():
    nc = bacc.Bacc(None, target_bir_lowering=False, debug=False)
    with tile.TileContext(nc) as tc, ExitStack() as ctx:
        dram = ctx.enter_context(tc.tile_pool(name="dram", bufs=1, space="DRAM"))
        xb_d = dram.tile([CIN, HP * W], FP8, kind="ExternalInput", name="xb")
        xr_d = dram.tile([COUT, HB * W], BF16, kind="ExternalInput", name="xr")
        w1t_d = dram.tile([CIN, CMID], FP8, kind="ExternalInput", name="w1t")
        wrt_d = dram.tile([CMID, RED], BF16, kind="ExternalInput", name="wrt")
        wse_d = dram.tile([81, WCH * 128], FP8, kind="ExternalInput", name="wse")
        w3t_d = dram.tile([CMID, COUT], BF16, kind="ExternalInput", name="w3t")
        vec_d = dram.tile([128, 8], F32, kind="ExternalInput", name="vecs")
        ones_d = dram.tile([1, 2 * NF], FP8, kind="ExternalInput", name="ones")
        id_d = dram.tile([128, 128], BF16, kind="ExternalInput", name="ident")
        id2_d = dram.tile([128, CMID], BF16, kind="ExternalInput", name="ident2")
        y_d = dram.tile([COUT, HB * W], BF16, kind="ExternalOutput", name="y")

        # x halo load in 4 chunks matching conv1's j-loop. Descriptor
        # generation is serial per sequencer (~6us per chunk), so the four
        # chunks are issued from four different engine sequencers and their
        # generation overlaps; chunk 1 (conv1's first) goes first on sync.
        wpool = ctx.enter_context(tc.tile_pool(name="weights", bufs=1))
        xpool = ctx.enter_context(tc.tile_pool(name="xin", bufs=1))
        # x arrives fp8 WITHOUT the 7-col zero pads (401KB instead of 1MB):
        # w1t is host-scaled x16 (fp8 denormal range) with 1/16 folded into
        # the conv1 activation scale; the pad columns are memset to zero
        # (b1=0 so relu(b1)=0 matches the reference's zero-padded taps)
        # x loads as TWO channel-half DMAs (1568B contiguous runs, 128
        # descriptors each — the 4-pixel-chunk split had 392B runs and was
        # descriptor-overhead-bound). conv1's first matmul of each chunk
        # pair only needs channel-half 0.
        XCH = HH * W  # 392 px per conv1 chunk (7 rows x 56 cols)
        xb = xpool.tile([128, 2, HP * W], FP8)
        xb_src = xb_d[:].rearrange("(c p) n -> p c n", p=128)
        # the three transfers gating conv1's first matmuls (x center rows
        # 7..20 per channel half + w1t) issue from THREE different
        # sequencers so their descriptor-gen + queue latencies overlap
        nc.sync.dma_start(out=xb[:, 0, XCH:3 * XCH], in_=xb_src[:, 0, XCH:3 * XCH])
        w1t = wpool.tile([128, 2, CMID], FP8)
        nc.scalar.dma_start(out=w1t[:], in_=w1t_d[:].rearrange("(c p) m -> p c m", p=128))
        nc.gpsimd.dma_start(out=xb[:, 1, XCH:3 * XCH], in_=xb_src[:, 1, XCH:3 * XCH])
        # small but gating transfers (vecs gates every conv1 relu, wrt the
        # reduce) go early so they don't queue behind the bulk transfers
        wrt = wpool.tile([CMID, RED], BF16)
        nc.sync.dma_start(out=wrt[:], in_=wrt_d[:])
        vecs = wpool.tile([128, 8], F32)
        nc.sync.dma_start(out=vecs[:], in_=vec_d[:])
        for cc, j in ((0, 0), (1, 0), (0, 3), (1, 3)):
            nc.sync.dma_start(out=xb[:, cc, ts(j, XCH)],
                              in_=xb_src[:, cc, ts(j, XCH)])
        # span weights in 3 vertical chunks at partitions 0/32/64; issued
        # from the scalar sequencer (below, after the ACT warm-up)
        wse = wpool.tile([81, WCH * 128], FP8)
        ident2 = wpool.tile([128, CMID], BF16)
        nc.gpsimd.dma_start(out=ident2[:], in_=id2_d[:])
        ident = wpool.tile([128, 128], BF16)
        nc.gpsimd.dma_start(out=ident[:], in_=id_d[:])
        w3t = wpool.tile([CMID, COUT], BF16)
        nc.gpsimd.dma_start(out=w3t[:], in_=w3t_d[:])
        xrb = xpool.tile([128, 2, HB * W], BF16)
        nc.gpsimd.dma_start(out=xrb[:], in_=xr_d[:].rearrange("(c p) n -> p c n", p=128))

        opool = ctx.enter_context(tc.tile_pool(name="out1", bufs=1))
        out1p = opool.tile([128, NP], BF16)
        out1q = opool.tile([128, NP], BF16)
        out1r = opool.tile([128, NP], BF16)  # row-14 pairs: [0:64]=blockA, [64:128]=blockA<<1col

        spool = ctx.enter_context(tc.tile_pool(name="stage", bufs=1))
        r_sb = spool.tile([81, 2 * NF], FP8)
        # ones row (span bias via the K dimension); DMA since engines cannot
        # address a single partition at offset 16
        nc.sync.dma_start(out=r_sb[RED:RED + 1, :], in_=ones_d[:])

        o3 = out1p[:].rearrange("p (h w) -> p h w", w=WP)

        # Pre-warm while the x DMA is in flight: memset-sourced dummy matmuls
        # ramp the PE p-state, and a dummy Relu pulls the 1.3us ACT table
        # load off conv1's critical path.
        warm_lhs = wpool.tile([128, CMID], BF16)
        warm_src = wpool.tile([128, 490], BF16)
        warm_out = wpool.tile([128, 1], F32)
        nc.vector.memset(warm_lhs[:], 0.25)
        nc.vector.memset(warm_src[:], 0.25)
        nc.scalar.activation(warm_out[:], warm_lhs[:, 0:1], AF.Relu, scale=1.0)
        # wse's descriptor generation rides the scalar sequencer after the
        # ACT warm-up (table load) but before conv1's first relu
        nc.scalar.dma_start(out=wse[:], in_=wse_d[:])
        with tc.tile_pool(name="pw", bufs=2, space="PSUM") as pw:
            for _i in range(4):
                wp_ = pw.tile([CMID, 490], F32, tag="warm")
                nc.tensor.matmul(wp_[:], warm_lhs[:], warm_src[:],
                                 start=True, stop=True)

        # zero the pad columns of out1p (rows x cols 0:7 and 63:70); the
        # +7-row dup copies them along. b1=0 -> relu(b1)=0 there, matching
        # the reference's zero-padded involution taps.
        nc.vector.memset(o3[0:CMID, :, 0:PAD], 0.0)
        nc.vector.memset(o3[0:CMID, :, PAD + W:WP], 0.0)

        # conv1 chunks 1,2 first, then the reduce (which only needs central
        # rows 7..20 = those chunks), then chunk 0 (the first involution
        # window needs rows 0..20, not chunk 3), then chunk 3.
        with tc.tile_pool(name="p1", bufs=4, space="PSUM") as p1, \
             tc.tile_pool(name="pr", bufs=2, space="PSUM") as pr:
            def conv1_chunk(j):
                ps = p1.tile([CMID, XCH], F32, tag="ps1")
                nc.tensor.matmul(ps[:], w1t[:, 0, :], xb[:, 0, ts(j, XCH)],
                                 start=True, stop=False)
                nc.tensor.matmul(ps[:], w1t[:, 1, :], xb[:, 1, ts(j, XCH)],
                                 start=False, stop=True)
                nc.scalar.activation(
                    o3[0:CMID, HH * j:HH * (j + 1), PAD:PAD + W], ps[:],
                    AF.Relu, bias=vecs[0:CMID, 1:2], scale=vecs[0:CMID, 0:1])

            conv1_chunk(1)
            conv1_chunk(2)
            for hhalf in range(2):
                ps = pr.tile([RED, NF], F32, tag="psr")
                nc.tensor.matmul(ps[:], wrt[:],
                                 o3[0:CMID, PAD + HH * hhalf:PAD + HH * (hhalf + 1), PAD:PAD + W],
                                 start=True, stop=True)
                nc.scalar.activation(r_sb[0:RED, ts(hhalf, NF)], ps[:], AF.Relu,
                                     bias=vecs[0:RED, 3:4], scale=vecs[0:RED, 2:3])
                # replicate this half of r (+ones row) at partitions 32/64
                # right away: the span's first matmul only needs half 0
                for c in range(1, 3):
                    nc.sync.dma_start(
                        out=r_sb[32 * c:32 * c + RED + 1, ts(hhalf, NF)],
                        in_=r_sb[0:RED + 1, ts(hhalf, NF)])
            conv1_chunk(0)
            conv1_chunk(3)

        # rows 7..27 duplicated at partitions 64:128 (the +7-row tap shift);
        # zero the unwritten tail there so zero-padded taps read 0, not junk.
        # The two big shift DMAs go out on the tensor/gpsimd sequencers so
        # their ~5us descriptor generation overlaps conv1 compute.
        DUPW = (HP - HH) * WP  # 1470
        nc.vector.memset(out1p[CMID:128, DUPW:NP], 0.0)
        nc.vector.memset(out1q[CMID:128, DUPW - 1:NP], 0.0)
        nc.vector.memset(out1q[0:CMID, NP - 1:NP], 0.0)
        # dup split: rows 7..20 (conv1 chunks 1,2) first — that is all the
        # first ky=0 quads need; rows 21..27 (chunk 3) follow
        DUPA = (HB - HH) * WP  # 980: dup rows 0..13 = orig rows 7..20
        nc.gpsimd.dma_start(out=out1p[CMID:128, 0:DUPA],
                            in_=out1p[0:CMID, HH * WP:HH * WP + DUPA])
        nc.gpsimd.dma_start(out=out1p[CMID:128, DUPA:DUPW],
                            in_=out1p[0:CMID, HH * WP + DUPA:NP])
        o3q = out1q[:].rearrange("p (h w) -> p h w", w=WP)
        o3r = out1r[:].rearrange("p (h w) -> p h w", w=WP)
        nc.vector.memset(out1r[CMID:128, NP - 1:NP], 0.0)
        nc.gpsimd.dma_start(out=out1r[CMID:128, 0:NP - 1],
                            in_=out1p[0:CMID, 1:NP])

        def emit_shift_copies():
            # +1-col shifted copies (odd-kx 4B alignment) are same-partition,
            # so they run on DVE; emitted AFTER the first four (even-kx)
            # quads' muls so DVE starts multiplying as early as possible
            nc.vector.tensor_copy(out1q[0:CMID, 0:NP - 1], out1p[0:CMID, 1:NP])
            nc.vector.tensor_copy(out1q[CMID:128, 0:DUPW - 1],
                                  out1p[CMID:128, 1:DUPW])
            # row-14 pair source: partitions 64:128 hold block A shifted +1 col
            nc.vector.tensor_copy(out1r[0:CMID, :], out1p[0:CMID, :])

        def window(pi):
            """shifted out1 window for pair pi as a [128, 2, 7, 56] view"""
            if pi < 7 * K:
                ky, kx = pi // K, pi % K
                if kx % 2 == 0:
                    src_ = o3[:, ky:ky + 2 * HH, kx:kx + W]
                else:
                    src_ = o3q[:, ky:ky + 2 * HH, kx - 1:kx - 1 + W]
            else:
                kx = 2 * (pi - 7 * K)  # row-14 pair (14,kx)+(14,kx+1)
                src_ = o3r[:, 14:14 + 2 * HH, kx:kx + W]
            return src_.rearrange("p (b h) w -> p b h w", b=2)

        def fused_window(qa, qb):
            """[128][2 pairs][14 rows][56 cols] window for the quad
            (qa, qb): +2 cols for same-ky quads, +1 row for cross-ky"""
            if qb == qa + 15:  # cross-ky kx=13 pair: pair dim steps one row
                ky, kx = qa // K, qa % K
                return _fused4d(out1q, ky * WP + kx - 1, WP, WP)
            if qa < 7 * K:
                ky, kx = qa // K, qa % K
                t_, base = (out1p, ky * WP + kx) if kx % 2 == 0 else \
                           (out1q, ky * WP + kx - 1)
            else:
                t_, base = out1r, 14 * WP + 2 * (qa - 7 * K)
            return _fused4d(t_, base, 2, WP)

        # involution span: same-parity quads, fused muls, two incremental
        # accumulation chains (DVE + the otherwise-idle GpSimd)
        quanta = []
        for ky in range(7):
            bq = 15 * ky
            for a, c in ((0, 2), (4, 6), (8, 10), (12, 14),
                         (1, 3), (5, 7), (9, 11)):
                quanta.append((bq + a, bq + c))
        for t4 in range(4):
            quanta.append((105 + 2 * t4, 106 + 2 * t4))
        # the per-ky kx=13 leftovers fuse ACROSS ky (windows one row apart)
        for kk in range(3):
            quanta.append((30 * kk + 13, 30 * kk + 28))
        # the solo pair goes mid-stream (NOT last): the final DVE op is then
        # quad 56's mul, and the already-ready accD/solo merge folds run on
        # the PE underneath the last few multiplies
        quanta.insert(44, (103, None))

        # Accumulation: DVE is the saturated engine (mul 884 + add 888 per
        # quad == the observed 1.77us/quad span rate), so the last TAIL
        # quads leave the DVE chain: their products are folded on the PE
        # (ident2 matmuls into ring banks 6/7, which double as the merge
        # accumulator) while DVE is still multiplying. After the final mul
        # only the solo fold + conv3 remain.
        TAIL0 = 53         # fq >= TAIL0 products are PE-merged, not chained
        PGRAN = 50         # fq >= PGRAN quads use pair-granular slots 0..5
        chain = dict(prev=None, acc=None)
        solo_prod = None
        fq = 0
        first_pm = [True, True]
        out2f = spool.tile([CMID, 2 * NF], BF16)
        with tc.tile_pool(name="sp", bufs=1, space="PSUM") as sp, \
             tc.tile_pool(name="we", bufs=3) as we_pool, \
             tc.tile_pool(name="prod", bufs=6) as prod_pool, \
             tc.tile_pool(name="accp", bufs=2) as acc_pool:
            ring = sp.tile([128, 4096], F32)   # 8 banks: ring + merge acc
            rv = ring[:].rearrange("p (s x) -> p s x", x=512)
            pmv = [ring[0:CMID, (6 + h) * 512:(6 + h) * 512 + NF]
                   for h in range(2)]

            def pm_fold(plane2, h, stop=False):
                nc.tensor.matmul(pmv[h], ident2[:], plane2,
                                 start=first_pm[h], stop=stop,
                                 skip_group_check=True)
                first_pm[h] = False

            cur = 0
            tcur = 0
            for jq, (qa, qb) in enumerate(quanta):
                if jq == 4:
                    emit_shift_copies()
                pairs = [qa] if qb is None else [qa, qb]
                if qb is not None:
                    fq += 1
                tail = fq >= PGRAN
                nsl = 2 * len(pairs)
                we4 = we_pool.tile([128, nsl, NF], BF16,
                                   tag="we4" if qb is not None else "we2")
                if not tail:
                    r0 = cur if cur + nsl <= 8 else 0
                    cur = (r0 + nsl) % 8
                    for idx, pi in enumerate(pairs):
                        wch, wo = pi // WCH, pi % WCH
                        lhsT = wse[32 * wch:32 * wch + RED + 1, ts(wo, 128)]
                        rr = r_sb[32 * wch:32 * wch + RED + 1, :]
                        for h in range(2):
                            sl = r0 + 2 * idx + h
                            nc.tensor.matmul(ring[:, sl * 512:sl * 512 + NF],
                                             lhsT, rr[:, ts(h, NF)],
                                             start=True, stop=True)
                    nc.scalar.activation(we4[:], rv[:, r0:r0 + nsl, 0:NF],
                                         AF.Copy, scale=1.0)

                else:
                    # slots 6/7 now hold the merge accumulator: pair-granular
                    # ring over slots 0..5 (3 pairs in flight)
                    for idx, pi in enumerate(pairs):
                        wch, wo = pi // WCH, pi % WCH
                        lhsT = wse[32 * wch:32 * wch + RED + 1, ts(wo, 128)]
                        rr = r_sb[32 * wch:32 * wch + RED + 1, :]
                        r0p = tcur
                        tcur = (tcur + 2) % 6
                        for h in range(2):
                            nc.tensor.matmul(
                                ring[:, (r0p + h) * 512:(r0p + h) * 512 + NF],
                                lhsT, rr[:, ts(h, NF)], start=True, stop=True)
                        nc.scalar.activation(we4[:, 2 * idx:2 * idx + 2, :],
                                             rv[:, r0p:r0p + 2, 0:NF],
                                             AF.Copy, scale=1.0)
                if qb is not None:
                    # one fused mul covers both pairs of the quad
                    prod2 = prod_pool.tile([128, 2, 2 * NF], BF16, tag="prod")
                    nc.vector.tensor_mul(
                        _fused4d(prod2[:].rearrange("p q n -> p (q n)"), 0,
                                 2 * NF, W),
                        _fused4d(we4[:].rearrange("p q n -> p (q n)"), 0,
                                 2 * NF, W),
                        fused_window(qa, qb))
                    if fq == 56:
                        last_q = prod2   # folds emitted after the loop (stop)
                    elif fq >= TAIL0:
                        # fold on the PE during the remaining span work
                        for h in range(2):
                            for q_ in range(2):
                                pm_fold(prod2[:, q_, ts(h, NF)], h)
                    elif chain["prev"] is None and chain["acc"] is None:
                        chain["prev"] = prod2
                    elif chain["acc"] is None:
                        d = acc_pool.tile([128, 2, 2 * NF], BF16, tag="acc")
                        nc.vector.tensor_add(d[:], chain["prev"][:], prod2[:])
                        chain["acc"] = d
                        chain["prev"] = None
                    else:
                        na = acc_pool.tile([128, 2, 2 * NF], BF16, tag="acc")
                        nc.vector.tensor_add(na[:], chain["acc"][:], prod2[:])
                        chain["acc"] = na
                else:  # solo pair: mul now, folded right below
                    ps1 = prod_pool.tile([128, 2 * NF], BF16, tag="psolo")
                    nc.vector.tensor_mul(
                        ps1[:].rearrange("p (b h w) -> p b h w", b=2, w=W),
                        we4[:].rearrange("p b (h w) -> p b h w", w=W),
                        window(qa))
                    solo_prod = ps1

            # remaining merge planes: the chain accumulator and solo product
            # (ready early, folded under the last muls), then quad 56's
            # product closes both banks; ACT applies gamma2/+b2/ReLU straight
            # from the PSUM accumulator.
            accD = chain["acc"]
            for h in range(2):
                pm_fold(accD[:, 0, ts(h, NF)], h)
                pm_fold(accD[:, 1, ts(h, NF)], h)
                pm_fold(solo_prod[:, ts(h, NF)], h)
            for h in range(2):
                pm_fold(last_q[:, 0, ts(h, NF)], h)
                pm_fold(last_q[:, 1, ts(h, NF)], h, stop=True)
            for h in range(2):
                nc.scalar.activation(out2f[:, ts(h, NF)], pmv[h], AF.Relu,
                                     bias=vecs[0:CMID, 4:5],
                                     scale=vecs[0:CMID, 7:8])

        # conv3 (g3 pre-folded into W3) + residual via identity matmul into
        # PSUM + bias b3 + relu; bf16 output DMA. Output DMAs issue from two
        # sequencers; their descriptor generation happened early (the
        # dma_start instructions pre-generate, then wait on the yr semaphore).
        y_dst = y_d[:].rearrange("(c p) n -> p c n", p=128)
        y_eng = {(0, 0): nc.sync, (0, 1): nc.gpsimd,
                 (1, 0): nc.sync, (1, 1): nc.gpsimd}
        with tc.tile_pool(name="p3", bufs=2, space="PSUM") as p3, \
             tc.tile_pool(name="ypool", bufs=2) as ypool:
            for nh in range(2):
                for mc in range(2):
                    ps = p3.tile([128, NF], F32, tag="ps3")
                    nc.tensor.matmul(ps[:], w3t[:, ts(mc, 128)], out2f[:, ts(nh, NF)],
                                     start=True, stop=False)
                    nc.tensor.matmul(ps[:], ident[:], xrb[:, mc, ts(nh, NF)],
                                     start=False, stop=True)
                    yr = ypool.tile([128, NF], BF16, tag="yr")
                    nc.scalar.activation(yr[:], ps[:], AF.Relu,
                                         bias=vecs[:, 5 + mc:6 + mc], scale=1.0)
                    y_eng[(nh, mc)].dma_start(
                        out=y_dst[:, mc, ts(nh, NF)], in_=yr[:])

    nc.compile()
    names = dict(xb=xb_d.name, xr=xr_d.name, w1t=w1t_d.name, wrt=wrt_d.name,
                 wse=wse_d.name, w3t=w3t_d.name, vecs=vec_d.name,
                 ones=ones_d.name, ident=id_d.name, ident2=id2_d.name, y=y_d.name)
    return nc, names


def _get_program():
    global _PROGRAM
    if _PROGRAM is None:
        _PROGRAM = _build_program()
    return _PROGRAM


def _bf16(a):
    return np.asarray(a, dtype=np.float32).astype(ml_dtypes.bfloat16)


def _host_inputs(x, W1, g1, b1, Wr, gr, br, Ws, bs, g2, b2, W3, g3, b3, names):
    x = np.asarray(x, dtype=np.float32)
    # conv1 weights ship fp8 scaled x16 (their ~0.05 magnitudes live in
    # e4m3's denormal range); the 1/16 is folded into the conv1 scale
    w1t = (np.asarray(W1).T * 16.0).astype(ml_dtypes.float8_e4m3)  # [256, 64]
    wrt = _bf16(np.asarray(Wr).T)                      # [64, 16]
    # fold BN3 gamma into W3 (rows scaled per output channel)
    w3t = _bf16((np.asarray(g3)[:, None] * np.asarray(W3)).T)  # [64, 256]

    # span weights, 16x channel-expanded, tap-paired (ky, ky+7), bias row 16.
    # BN2 gamma is NOT folded here: the merge activation applies it as a
    # per-partition scale.
    Ws = np.asarray(Ws, dtype=np.float32)              # [900, 16]
    bs = np.asarray(bs, dtype=np.float32)              # [900]
    g2 = np.asarray(g2, dtype=np.float32)              # [64]
    gidx = np.arange(CMID) // GC                       # [64]
    WsT = Ws.reshape(G, K * K, RED)                    # [g, k, rho]
    bsr = bs.reshape(G, K * K)
    wse = np.zeros((RED + 1, NPAIR, 128), dtype=np.float32)
    for pi in range(NPAIR):
        if pi < 7 * K:
            ky, kx = pi // K, pi % K
            k1, k2 = ky * K + kx, (ky + 7) * K + kx
        else:
            kx = 2 * (pi - 7 * K)
            k1 = 14 * K + kx
            k2 = 14 * K + kx + 1 if kx + 1 < K else None
        wse[0:RED, pi, 0:CMID] = WsT[gidx, k1, :].T
        wse[RED, pi, 0:CMID] = bsr[gidx, k1]
        if k2 is not None:
            wse[0:RED, pi, CMID:128] = WsT[gidx, k2, :].T
            wse[RED, pi, CMID:128] = bsr[gidx, k2]
    wse4 = np.zeros((81, WCH * 128), dtype=np.float32)
    for pi in range(NPAIR):
        wc, wo = pi // WCH, pi % WCH
        wse4[32 * wc:32 * wc + RED + 1, wo * 128:(wo + 1) * 128] = wse[:, pi, :]
    wse = np.asarray(wse4, dtype=np.float32).astype(ml_dtypes.float8_e4m3)

    vecs = np.zeros((128, 8), dtype=np.float32)
    vecs[0:CMID, 0] = np.asarray(g1) / 16.0
    vecs[0:CMID, 1] = b1
    vecs[0:RED, 2] = gr
    vecs[0:RED, 3] = br
    vecs[0:CMID, 4] = b2
    vecs[:, 5] = np.asarray(b3)[0:128]
    vecs[:, 6] = np.asarray(b3)[128:256]
    vecs[0:CMID, 7] = g2

    ident = np.eye(128, dtype=np.float32)
    ident2 = np.zeros((128, CMID), dtype=np.float32)
    ident2[np.arange(128), np.arange(128) % CMID] = 1.0

    in_maps = []
    core_geom = []
    for core in range(8):
        b = core // 4
        h0 = (core % 4) * HB
        xpad = np.zeros((CIN, HP, W), dtype=np.float32)
        lo, hi = h0 - PAD, h0 + HB + PAD
        slo, shi = max(lo, 0), min(hi, H)
        xpad[:, slo - lo:shi - lo, :] = x[b, :, slo:shi, :]
        xbc = xpad.reshape(CIN, HP * W).astype(ml_dtypes.float8_e4m3)
        xrc = _bf16(np.ascontiguousarray(x[b, :, h0:h0 + HB, :]).reshape(COUT, HB * W))
        in_maps.append({
            names["xb"]: xbc,
            names["xr"]: xrc,
            names["w1t"]: w1t,
            names["wrt"]: wrt,
            names["wse"]: wse,
            names["w3t"]: w3t,
            names["vecs"]: vecs,
            names["ones"]: np.ones((1, 2 * NF), dtype=np.float32).astype(ml_dtypes.float8_e4m3),
            names["ident"]: _bf16(ident),
            names["ident2"]: _bf16(ident2),
        })
        core_geom.append((b, h0))
    return in_maps, core_geom


def kernel(x, W1, g1, b1, Wr, gr, br, Ws, bs, g2, b2, W3, g3, b3,
           _want_results=False, _trace=False):
    nc, names = _get_program()
    in_maps, core_geom = _host_inputs(x, W1, g1, b1, Wr, gr, br, Ws, bs,
                                      g2, b2, W3, g3, b3, names)

    res = run_bass_kernel_spmd(nc, in_maps, list(range(8)), trace=_trace)

    y = np.empty((B, COUT, H, W), dtype=np.float32)
    for core, (b, h0) in enumerate(core_geom):
        y[b, :, h0:h0 + HB, :] = np.asarray(
            res.results[core][names["y"]], dtype=np.float32).reshape(COUT, HB, W)
    if _want_results:
        return y, res
    return y


# revision 76
# speedup vs baseline: 1.1901x; 1.1901x over previous
"""Trainium2 Bass kernel for an involution Bottleneck block (B=2, Cin=256,
Cmid=64, Cout=256, H=W=56, K=15, G=4).

Sharding: 8 cores = 2 batches x 4 H-quarters (14 output rows each). Each core
receives a zero-padded input halo [256, 28, 70] (7 rows/cols each side), so no
inter-core communication is needed (halo compute is redundant).

Per-core pipeline (channels on SBUF partitions, pixels on free dim):
  conv1 1x1 (PE, bf16) -> BN+ReLU (ACT) -> out1 [64, 28x70] bf16, plus a copy
    shifted by +7 rows at partitions 64:128 and a +1-col copy (odd-kx align).
  involution as 113 tap-PAIRS (ky, ky+7): per pair one span matmul with M=128
    (2 taps x 64 group-expanded channels, bias via the ones-row at K=17) into
    an 8-bank PSUM ring.  Pairs are processed as same-parity-kx QUADS: ACT
    evicts the quad to bf16 SBUF, then ONE fused DVE tensor_mul covers both
    pairs.  ~6 "direct" quads skip the eviction: DVE multiplies straight from
    PSUM fp32 (1x DVE rate but zero ACT cost).  Accumulation runs as two
    incremental chains: DVE (fast) and GpSimd (slow but otherwise idle).
  merge on the PE (ident2 fold of the 2-tap partitions + all chain planes into
    PSUM) -> gamma2 scale + b2 + ReLU (ACT, per-partition scale) -> conv3 1x1
    with g3 folded into W3 (PE) accumulating the bf16 residual via an identity
    matmul -> +b3, ReLU (ACT) -> bf16 DMA out (host converts to f32).

DMA descriptor generation is ~25ns/row and serial per sequencer (~6us per
input-sized dma_start), so input loads and output stores are issued from
different engine sequencers to overlap their descriptor generation.
"""

import sys, types
sys.path.insert(0, "/opt/trn_rl_repo")

import numpy as np
import ml_dtypes
from contextlib import ExitStack

import concourse.bass as bass
import concourse.mybir as mybir
import concourse.tile as tile
from concourse import bacc
from concourse.bass import ts
from concourse.bass_utils import run_bass_kernel_spmd

BF16 = mybir.dt.bfloat16
FP8 = mybir.dt.float8e4
F32 = mybir.dt.float32
AF = mybir.ActivationFunctionType

K = 15
G = 4
GC = 16
PAD = 7
CIN = 256
CMID = 64
RED = 16
COUT = 256
H = 56
W = 56
B = 2
HB = 14            # output rows per core
HP = HB + 2 * PAD  # 28 padded rows
WP = W + 2 * PAD   # 70 padded cols
NP = HP * WP       # 1960
HH = HB // 2       # 7 rows per half-block
NF = HH * W        # 392 pixels per half-block
NPAIR = 7 * K + 8  # 105 (ky,ky+7) pairs + 8 row-14 (kx,kx+1) pairs = 113
WCH = 38           # wse pairs per 32-partition chunk

_PROGRAM = None  # (nc, names) cache


def _fused4d(ap_like, base, pair_stride, row_stride):
    """4D AP [128][2 pairs][14 rows][56 cols] at arbitrary strides."""
    s_ = ap_like[:, base:base + 1]
    return bass.AP(tensor=s_.tensor, offset=s_.offset,
                   ap=[list(s_.ap[0]), [pair_stride, 2], [row_stride, 14],
                       [1, 56]])


def _build_program():
    nc = bacc.Bacc(None, target_bir_lowering=False, debug=False)
    with tile.TileContext(nc) as tc, ExitStack() as ctx:
        dram = ctx.enter_context(tc.tile_pool(name="dram", bufs=1, space="DRAM"))
        xb_d = dram.tile([CIN, HP * W], FP8, kind="ExternalInput", name="xb")
        xr_d = dram.tile([COUT, HB * W], BF16, kind="ExternalInput", name="xr")
        w1t_d = dram.tile([CIN, CMID], FP8, kind="ExternalInput", name="w1t")
        wrt_d = dram.tile([CMID, RED], BF16, kind="ExternalInput", name="wrt")
        wse_d = dram.tile([81, WCH * 128], FP8, kind="ExternalInput", name="wse")
        w3t_d = dram.tile([CMID, COUT], BF16, kind="ExternalInput", name="w3t")
        vec_d = dram.tile([128, 8], F32, kind="ExternalInput", name="vecs")
        ones_d = dram.tile([1, 2 * NF], FP8, kind="ExternalInput", name="ones")
        id_d = dram.tile([128, 128], BF16, kind="ExternalInput", name="ident")
        id2_d = dram.tile([128, CMID], BF16, kind="ExternalInput", name="ident2")
        y_d = dram.tile([COUT, HB * W], BF16, kind="ExternalOutput", name="y")

        # x halo load in 4 chunks matching conv1's j-loop. Descriptor
        # generation is serial per sequencer (~6us per chunk), so the four
        # chunks are issued from four different engine sequencers and their
        # generation overlaps; chunk 1 (conv1's first) goes first on sync.
        wpool = ctx.enter_context(tc.tile_pool(name="weights", bufs=1))
        xpool = ctx.enter_context(tc.tile_pool(name="xin", bufs=1))
        # x arrives fp8 WITHOUT the 7-col zero pads (401KB instead of 1MB):
        # w1t is host-scaled x16 (fp8 denormal range) with 1/16 folded into
        # the conv1 activation scale; the pad columns are memset to zero
        # (b1=0 so relu(b1)=0 matches the reference's zero-padded taps)
        # x loads as TWO channel-half DMAs (1568B contiguous runs, 128
        # descriptors each — the 4-pixel-chunk split had 392B runs and was
        # descriptor-overhead-bound). conv1's first matmul of each chunk
        # pair only needs channel-half 0.
        XCH = HH * W  # 392 px per conv1 chunk (7 rows x 56 cols)
        xb = xpool.tile([128, 2, HP * W], FP8)
        xb_src = xb_d[:].rearrange("(c p) n -> p c n", p=128)
        # center rows 7..20 (conv1 chunks 1+2, feeding the reduce) first,
        # per channel half; then the halo chunks 0 and 3.
        # NOTE: issuing w1t/B-half from the scalar/gpsimd sequencers was
        # measured to flip the chip into a globally ~20% slower state
        # (every engine's op latency rises) — keep these on sync.
        nc.sync.dma_start(out=xb[:, 0, XCH:3 * XCH], in_=xb_src[:, 0, XCH:3 * XCH])
        w1t = wpool.tile([128, 2, CMID], FP8)
        nc.sync.dma_start(out=w1t[:], in_=w1t_d[:].rearrange("(c p) m -> p c m", p=128))
        nc.sync.dma_start(out=xb[:, 1, XCH:3 * XCH], in_=xb_src[:, 1, XCH:3 * XCH])
        # small but gating transfers (vecs gates every conv1 relu, wrt the
        # reduce) go early so they don't queue behind the bulk transfers
        wrt = wpool.tile([CMID, RED], BF16)
        nc.sync.dma_start(out=wrt[:], in_=wrt_d[:])
        vecs = wpool.tile([128, 8], F32)
        nc.sync.dma_start(out=vecs[:], in_=vec_d[:])
        for cc, j in ((0, 0), (1, 0), (0, 3), (1, 3)):
            nc.sync.dma_start(out=xb[:, cc, ts(j, XCH)],
                              in_=xb_src[:, cc, ts(j, XCH)])
        # span weights in 3 vertical chunks at partitions 0/32/64; issued
        # from the scalar sequencer (below, after the ACT warm-up)
        wse = wpool.tile([81, WCH * 128], FP8)
        ident2 = wpool.tile([128, CMID], BF16)
        nc.gpsimd.dma_start(out=ident2[:], in_=id2_d[:])
        ident = wpool.tile([128, 128], BF16)
        nc.gpsimd.dma_start(out=ident[:], in_=id_d[:])
        w3t = wpool.tile([CMID, COUT], BF16)
        nc.gpsimd.dma_start(out=w3t[:], in_=w3t_d[:])
        xrb = xpool.tile([128, 2, HB * W], BF16)
        nc.gpsimd.dma_start(out=xrb[:], in_=xr_d[:].rearrange("(c p) n -> p c n", p=128))

        opool = ctx.enter_context(tc.tile_pool(name="out1", bufs=1))
        out1p = opool.tile([128, NP], BF16)
        out1q = opool.tile([128, NP], BF16)
        out1r = opool.tile([128, NP], BF16)  # row-14 pairs: [0:64]=blockA, [64:128]=blockA<<1col

        spool = ctx.enter_context(tc.tile_pool(name="stage", bufs=1))
        r_sb = spool.tile([81, 2 * NF], FP8)
        # ones row (span bias via the K dimension); DMA since engines cannot
        # address a single partition at offset 16
        nc.sync.dma_start(out=r_sb[RED:RED + 1, :], in_=ones_d[:])

        o3 = out1p[:].rearrange("p (h w) -> p h w", w=WP)

        # Pre-warm while the x DMA is in flight: memset-sourced dummy matmuls
        # ramp the PE p-state, and a dummy Relu pulls the 1.3us ACT table
        # load off conv1's critical path.
        warm_lhs = wpool.tile([128, CMID], BF16)
        warm_src = wpool.tile([128, 490], BF16)
        warm_out = wpool.tile([128, 1], F32)
        nc.vector.memset(warm_lhs[:], 0.25)
        nc.vector.memset(warm_src[:], 0.25)
        nc.scalar.activation(warm_out[:], warm_lhs[:, 0:1], AF.Relu, scale=1.0)
        # wse's descriptor generation rides the scalar sequencer after the
        # ACT warm-up (table load) but before conv1's first relu
        nc.scalar.dma_start(out=wse[:], in_=wse_d[:])
        with tc.tile_pool(name="pw", bufs=2, space="PSUM") as pw:
            for _i in range(4):
                wp_ = pw.tile([CMID, 490], F32, tag="warm")
                nc.tensor.matmul(wp_[:], warm_lhs[:], warm_src[:],
                                 start=True, stop=True)

        # zero the pad columns of out1p (rows x cols 0:7 and 63:70); the
        # +7-row dup copies them along. b1=0 -> relu(b1)=0 there, matching
        # the reference's zero-padded involution taps.
        nc.vector.memset(o3[0:CMID, :, 0:PAD], 0.0)
        nc.vector.memset(o3[0:CMID, :, PAD + W:WP], 0.0)

        # conv1 chunks 1,2 first, then the reduce (which only needs central
        # rows 7..20 = those chunks), then chunk 0 (the first involution
        # window needs rows 0..20, not chunk 3), then chunk 3.
        with tc.tile_pool(name="p1", bufs=4, space="PSUM") as p1, \
             tc.tile_pool(name="pr", bufs=2, space="PSUM") as pr:
            def conv1_chunk(j):
                ps = p1.tile([CMID, XCH], F32, tag="ps1")
                nc.tensor.matmul(ps[:], w1t[:, 0, :], xb[:, 0, ts(j, XCH)],
                                 start=True, stop=False)
                nc.tensor.matmul(ps[:], w1t[:, 1, :], xb[:, 1, ts(j, XCH)],
                                 start=False, stop=True)
                nc.scalar.activation(
                    o3[0:CMID, HH * j:HH * (j + 1), PAD:PAD + W], ps[:],
                    AF.Relu, bias=vecs[0:CMID, 1:2], scale=vecs[0:CMID, 0:1])

            conv1_chunk(1)
            conv1_chunk(2)
            for hhalf in range(2):
                ps = pr.tile([RED, NF], F32, tag="psr")
                nc.tensor.matmul(ps[:], wrt[:],
                                 o3[0:CMID, PAD + HH * hhalf:PAD + HH * (hhalf + 1), PAD:PAD + W],
                                 start=True, stop=True)
                nc.scalar.activation(r_sb[0:RED, ts(hhalf, NF)], ps[:], AF.Relu,
                                     bias=vecs[0:RED, 3:4], scale=vecs[0:RED, 2:3])
                # replicate this half of r (+ones row) at partitions 32/64
                # right away: the span's first matmul only needs half 0
                for c in range(1, 3):
                    nc.sync.dma_start(
                        out=r_sb[32 * c:32 * c + RED + 1, ts(hhalf, NF)],
                        in_=r_sb[0:RED + 1, ts(hhalf, NF)])
            conv1_chunk(0)
            conv1_chunk(3)

        # rows 7..27 duplicated at partitions 64:128 (the +7-row tap shift);
        # zero the unwritten tail there so zero-padded taps read 0, not junk.
        # The two big shift DMAs go out on the tensor/gpsimd sequencers so
        # their ~5us descriptor generation overlaps conv1 compute.
        DUPW = (HP - HH) * WP  # 1470
        nc.vector.memset(out1p[CMID:128, DUPW:NP], 0.0)
        nc.vector.memset(out1q[CMID:128, DUPW - 1:NP], 0.0)
        nc.vector.memset(out1q[0:CMID, NP - 1:NP], 0.0)
        # dup split: rows 7..20 (conv1 chunks 1,2) first — that is all the
        # first ky=0 quads need; rows 21..27 (chunk 3) follow
        DUPA = (HB - HH) * WP  # 980: dup rows 0..13 = orig rows 7..20
        nc.gpsimd.dma_start(out=out1p[CMID:128, 0:DUPA],
                            in_=out1p[0:CMID, HH * WP:HH * WP + DUPA])
        nc.gpsimd.dma_start(out=out1p[CMID:128, DUPA:DUPW],
                            in_=out1p[0:CMID, HH * WP + DUPA:NP])
        o3q = out1q[:].rearrange("p (h w) -> p h w", w=WP)
        o3r = out1r[:].rearrange("p (h w) -> p h w", w=WP)
        nc.vector.memset(out1r[CMID:128, NP - 1:NP], 0.0)
        nc.gpsimd.dma_start(out=out1r[CMID:128, 0:NP - 1],
                            in_=out1p[0:CMID, 1:NP])

        def emit_shift_copies():
            # +1-col shifted copies (odd-kx 4B alignment) are same-partition,
            # so they run on DVE; emitted AFTER the first four (even-kx)
            # quads' muls so DVE starts multiplying as early as possible
            nc.vector.tensor_copy(out1q[0:CMID, 0:NP - 1], out1p[0:CMID, 1:NP])
            nc.vector.tensor_copy(out1q[CMID:128, 0:DUPW - 1],
                                  out1p[CMID:128, 1:DUPW])
            # row-14 pair source: partitions 64:128 hold block A shifted +1 col
            nc.vector.tensor_copy(out1r[0:CMID, :], out1p[0:CMID, :])

        def window(pi):
            """shifted out1 window for pair pi as a [128, 2, 7, 56] view"""
            if pi < 7 * K:
                ky, kx = pi // K, pi % K
                if kx % 2 == 0:
                    src_ = o3[:, ky:ky + 2 * HH, kx:kx + W]
                else:
                    src_ = o3q[:, ky:ky + 2 * HH, kx - 1:kx - 1 + W]
            else:
                kx = 2 * (pi - 7 * K)  # row-14 pair (14,kx)+(14,kx+1)
                src_ = o3r[:, 14:14 + 2 * HH, kx:kx + W]
            return src_.rearrange("p (b h) w -> p b h w", b=2)

        def fused_window(qa, qb):
            """[128][2 pairs][14 rows][56 cols] window for the quad
            (qa, qb): +2 cols for same-ky quads, +1 row for cross-ky"""
            if qb == qa + 15:  # cross-ky kx=13 pair: pair dim steps one row
                ky, kx = qa // K, qa % K
                return _fused4d(out1q, ky * WP + kx - 1, WP, WP)
            if qa < 7 * K:
                ky, kx = qa // K, qa % K
                t_, base = (out1p, ky * WP + kx) if kx % 2 == 0 else \
                           (out1q, ky * WP + kx - 1)
            else:
                t_, base = out1r, 14 * WP + 2 * (qa - 7 * K)
            return _fused4d(t_, base, 2, WP)

        # involution span: same-parity quads, fused muls, two incremental
        # accumulation chains (DVE + the otherwise-idle GpSimd)
        quanta = []
        for ky in range(7):
            bq = 15 * ky
            for a, c in ((0, 2), (4, 6), (8, 10), (12, 14),
                         (1, 3), (5, 7), (9, 11)):
                quanta.append((bq + a, bq + c))
        for t4 in range(4):
            quanta.append((105 + 2 * t4, 106 + 2 * t4))
        # the per-ky kx=13 leftovers fuse ACROSS ky (windows one row apart)
        for kk in range(3):
            quanta.append((30 * kk + 13, 30 * kk + 28))
        # the solo pair goes mid-stream (NOT last): the final DVE op is then
        # quad 56's mul, and the already-ready accD/solo merge folds run on
        # the PE underneath the last few multiplies
        quanta.insert(44, (103, None))

        # Accumulation: DVE is the saturated engine (mul 884 + add 888 per
        # quad == the observed 1.77us/quad span rate), so the last TAIL
        # quads leave the DVE chain: their products are folded on the PE
        # (ident2 matmuls into ring banks 6/7, which double as the merge
        # accumulator) while DVE is still multiplying. After the final mul
        # only the solo fold + conv3 remain.
        TAIL0 = 53         # fq >= TAIL0 products are PE-merged, not chained
        PGRAN = 50         # fq >= PGRAN quads use pair-granular slots 0..5
        chain = dict(prev=None, acc=None)
        solo_prod = None
        fq = 0
        first_pm = [True, True]
        out2f = spool.tile([CMID, 2 * NF], BF16)
        with tc.tile_pool(name="sp", bufs=1, space="PSUM") as sp, \
             tc.tile_pool(name="we", bufs=3) as we_pool, \
             tc.tile_pool(name="prod", bufs=6) as prod_pool, \
             tc.tile_pool(name="accp", bufs=2) as acc_pool:
            ring = sp.tile([128, 4096], F32)   # 8 banks: ring + merge acc
            rv = ring[:].rearrange("p (s x) -> p s x", x=512)
            pmv = [ring[0:CMID, (6 + h) * 512:(6 + h) * 512 + NF]
                   for h in range(2)]

            def pm_fold(plane2, h, stop=False):
                nc.tensor.matmul(pmv[h], ident2[:], plane2,
                                 start=first_pm[h], stop=stop,
                                 skip_group_check=True)
                first_pm[h] = False

            cur = 0
            tcur = 0
            for jq, (qa, qb) in enumerate(quanta):
                if jq == 4:
                    emit_shift_copies()
                pairs = [qa] if qb is None else [qa, qb]
                if qb is not None:
                    fq += 1
                tail = fq >= PGRAN
                nsl = 2 * len(pairs)
                we4 = we_pool.tile([128, nsl, NF], BF16,
                                   tag="we4" if qb is not None else "we2")
                if not tail:
                    r0 = cur if cur + nsl <= 8 else 0
                    cur = (r0 + nsl) % 8
                    for idx, pi in enumerate(pairs):
                        wch, wo = pi // WCH, pi % WCH
                        lhsT = wse[32 * wch:32 * wch + RED + 1, ts(wo, 128)]
                        rr = r_sb[32 * wch:32 * wch + RED + 1, :]
                        for h in range(2):
                            sl = r0 + 2 * idx + h
                            nc.tensor.matmul(ring[:, sl * 512:sl * 512 + NF],
                                             lhsT, rr[:, ts(h, NF)],
                                             start=True, stop=True)
                    nc.scalar.activation(we4[:], rv[:, r0:r0 + nsl, 0:NF],
                                         AF.Copy, scale=1.0)

                else:
                    # slots 6/7 now hold the merge accumulator: pair-granular
                    # ring over slots 0..5 (3 pairs in flight)
                    for idx, pi in enumerate(pairs):
                        wch, wo = pi // WCH, pi % WCH
                        lhsT = wse[32 * wch:32 * wch + RED + 1, ts(wo, 128)]
                        rr = r_sb[32 * wch:32 * wch + RED + 1, :]
                        r0p = tcur
                        tcur = (tcur + 2) % 6
                        for h in range(2):
                            nc.tensor.matmul(
                                ring[:, (r0p + h) * 512:(r0p + h) * 512 + NF],
                                lhsT, rr[:, ts(h, NF)], start=True, stop=True)
                        nc.scalar.activation(we4[:, 2 * idx:2 * idx + 2, :],
                                             rv[:, r0p:r0p + 2, 0:NF],
                                             AF.Copy, scale=1.0)
                if qb is not None:
                    # one fused mul covers both pairs of the quad
                    prod2 = prod_pool.tile([128, 2, 2 * NF], BF16, tag="prod")
                    nc.vector.tensor_mul(
                        _fused4d(prod2[:].rearrange("p q n -> p (q n)"), 0,
                                 2 * NF, W),
                        _fused4d(we4[:].rearrange("p q n -> p (q n)"), 0,
                                 2 * NF, W),
                        fused_window(qa, qb))
                    if fq == 56:
                        last_q = prod2   # folds emitted after the loop (stop)
                    elif fq >= TAIL0:
                        # fold on the PE during the remaining span work
                        for h in range(2):
                            for q_ in range(2):
                                pm_fold(prod2[:, q_, ts(h, NF)], h)
                    elif chain["prev"] is None and chain["acc"] is None:
                        chain["prev"] = prod2
                    elif chain["acc"] is None:
                        d = acc_pool.tile([128, 2, 2 * NF], BF16, tag="acc")
                        nc.vector.tensor_add(d[:], chain["prev"][:], prod2[:])
                        chain["acc"] = d
                        chain["prev"] = None
                    else:
                        na = acc_pool.tile([128, 2, 2 * NF], BF16, tag="acc")
                        nc.vector.tensor_add(na[:], chain["acc"][:], prod2[:])
                        chain["acc"] = na
                else:  # solo pair: mul now, folded right below
                    ps1 = prod_pool.tile([128, 2 * NF], BF16, tag="psolo")
                    nc.vector.tensor_mul(
                        ps1[:].rearrange("p (b h w) -> p b h w", b=2, w=W),
                        we4[:].rearrange("p b (h w) -> p b h w", w=W),
                        window(qa))
                    solo_prod = ps1

            # remaining merge planes: the chain accumulator and solo product
            # (ready early, folded under the last muls), then quad 56's
            # product closes both banks; ACT applies gamma2/+b2/ReLU straight
            # from the PSUM accumulator.
            accD = chain["acc"]
            for h in range(2):
                pm_fold(accD[:, 0, ts(h, NF)], h)
                pm_fold(accD[:, 1, ts(h, NF)], h)
                pm_fold(solo_prod[:, ts(h, NF)], h)
            for h in range(2):
                pm_fold(last_q[:, 0, ts(h, NF)], h)
                pm_fold(last_q[:, 1, ts(h, NF)], h, stop=True)
            for h in range(2):
                nc.scalar.activation(out2f[:, ts(h, NF)], pmv[h], AF.Relu,
                                     bias=vecs[0:CMID, 4:5],
                                     scale=vecs[0:CMID, 7:8])

        # conv3 (g3 pre-folded into W3) + residual via identity matmul into
        # PSUM + bias b3 + relu; bf16 output DMA. Output DMAs issue from two
        # sequencers; their descriptor generation happened early (the
        # dma_start instructions pre-generate, then wait on the yr semaphore).
        y_dst = y_d[:].rearrange("(c p) n -> p c n", p=128)
        y_eng = {(0, 0): nc.sync, (0, 1): nc.gpsimd,
                 (1, 0): nc.sync, (1, 1): nc.gpsimd}
        with tc.tile_pool(name="p3", bufs=2, space="PSUM") as p3, \
             tc.tile_pool(name="ypool", bufs=2) as ypool:
            for nh in range(2):
                for mc in range(2):
                    ps = p3.tile([128, NF], F32, tag="ps3")
                    nc.tensor.matmul(ps[:], w3t[:, ts(mc, 128)], out2f[:, ts(nh, NF)],
                                     start=True, stop=False)
                    nc.tensor.matmul(ps[:], ident[:], xrb[:, mc, ts(nh, NF)],
                                     start=False, stop=True)
                    yr = ypool.tile([128, NF], BF16, tag="yr")
                    nc.scalar.activation(yr[:], ps[:], AF.Relu,
                                         bias=vecs[:, 5 + mc:6 + mc], scale=1.0)
                    y_eng[(nh, mc)].dma_start(
                        out=y_dst[:, mc, ts(nh, NF)], in_=yr[:])

    nc.compile()
    names = dict(xb=xb_d.name, xr=xr_d.name, w1t=w1t_d.name, wrt=wrt_d.name,
                 wse=wse_d.name, w3t=w3t_d.name, vecs=vec_d.name,
                 ones=ones_d.name, ident=id_d.name, ident2=id2_d.name, y=y_d.name)
    return nc, names


def _get_program():
    global _PROGRAM
    if _PROGRAM is None:
        _PROGRAM = _build_program()
    return _PROGRAM


def _bf16(a):
    return np.asarray(a, dtype=np.float32).astype(ml_dtypes.bfloat16)


def _host_inputs(x, W1, g1, b1, Wr, gr, br, Ws, bs, g2, b2, W3, g3, b3, names):
    x = np.asarray(x, dtype=np.float32)
    # conv1 weights ship fp8 scaled x16 (their ~0.05 magnitudes live in
    # e4m3's denormal range); the 1/16 is folded into the conv1 scale
    w1t = (np.asarray(W1).T * 16.0).astype(ml_dtypes.float8_e4m3)  # [256, 64]
    wrt = _bf16(np.asarray(Wr).T)                      # [64, 16]
    # fold BN3 gamma into W3 (rows scaled per output channel)
    w3t = _bf16((np.asarray(g3)[:, None] * np.asarray(W3)).T)  # [64, 256]

    # span weights, 16x channel-expanded, tap-paired (ky, ky+7), bias row 16.
    # BN2 gamma is NOT folded here: the merge activation applies it as a
    # per-partition scale.
    Ws = np.asarray(Ws, dtype=np.float32)              # [900, 16]
    bs = np.asarray(bs, dtype=np.float32)              # [900]
    g2 = np.asarray(g2, dtype=np.float32)              # [64]
    gidx = np.arange(CMID) // GC                       # [64]
    WsT = Ws.reshape(G, K * K, RED)                    # [g, k, rho]
    bsr = bs.reshape(G, K * K)
    wse = np.zeros((RED + 1, NPAIR, 128), dtype=np.float32)
    for pi in range(NPAIR):
        if pi < 7 * K:
            ky, kx = pi // K, pi % K
            k1, k2 = ky * K + kx, (ky + 7) * K + kx
        else:
            kx = 2 * (pi - 7 * K)
            k1 = 14 * K + kx
            k2 = 14 * K + kx + 1 if kx + 1 < K else None
        wse[0:RED, pi, 0:CMID] = WsT[gidx, k1, :].T
        wse[RED, pi, 0:CMID] = bsr[gidx, k1]
        if k2 is not None:
            wse[0:RED, pi, CMID:128] = WsT[gidx, k2, :].T
            wse[RED, pi, CMID:128] = bsr[gidx, k2]
    wse4 = np.zeros((81, WCH * 128), dtype=np.float32)
    for pi in range(NPAIR):
        wc, wo = pi // WCH, pi % WCH
        wse4[32 * wc:32 * wc + RED + 1, wo * 128:(wo + 1) * 128] = wse[:, pi, :]
    wse = np.asarray(wse4, dtype=np.float32).astype(ml_dtypes.float8_e4m3)

    vecs = np.zeros((128, 8), dtype=np.float32)
    vecs[0:CMID, 0] = np.asarray(g1) / 16.0
    vecs[0:CMID, 1] = b1
    vecs[0:RED, 2] = gr
    vecs[0:RED, 3] = br
    vecs[0:CMID, 4] = b2
    vecs[:, 5] = np.asarray(b3)[0:128]
    vecs[:, 6] = np.asarray(b3)[128:256]
    vecs[0:CMID, 7] = g2

    ident = np.eye(128, dtype=np.float32)
    ident2 = np.zeros((128, CMID), dtype=np.float32)
    ident2[np.arange(128), np.arange(128) % CMID] = 1.0

    in_maps = []
    core_geom = []
    for core in range(8):
        b = core // 4
        h0 = (core % 4) * HB
        xpad = np.zeros((CIN, HP, W), dtype=np.float32)
        lo, hi = h0 - PAD, h0 + HB + PAD
        slo, shi = max(lo, 0), min(hi, H)
        xpad[:, slo - lo:shi - lo, :] = x[b, :, slo:shi, :]
        xbc = xpad.reshape(CIN, HP * W).astype(ml_dtypes.float8_e4m3)
        xrc = _bf16(np.ascontiguousarray(x[b, :, h0:h0 + HB, :]).reshape(COUT, HB * W))
        in_maps.append({
            names["xb"]: xbc,
            names["xr"]: xrc,
            names["w1t"]: w1t,
            names["wrt"]: wrt,
            names["wse"]: wse,
            names["w3t"]: w3t,
            names["vecs"]: vecs,
            names["ones"]: np.ones((1, 2 * NF), dtype=np.float32).astype(ml_dtypes.float8_e4m3),
            names["ident"]: _bf16(ident),
            names["ident2"]: _bf16(ident2),
        })
        core_geom.append((b, h0))
    return in_maps, core_geom


def kernel(x, W1, g1, b1, Wr, gr, br, Ws, bs, g2, b2, W3, g3, b3,
           _want_results=False, _trace=False):
    nc, names = _get_program()
    in_maps, core_geom = _host_inputs(x, W1, g1, b1, Wr, gr, br, Ws, bs,
                                      g2, b2, W3, g3, b3, names)

    res = run_bass_kernel_spmd(nc, in_maps, list(range(8)), trace=_trace)

    y = np.empty((B, COUT, H, W), dtype=np.float32)
    for core, (b, h0) in enumerate(core_geom):
        y[b, :, h0:h0 + HB, :] = np.asarray(
            res.results[core][names["y"]], dtype=np.float32).reshape(COUT, HB, W)
    if _want_results:
        return y, res
    return y
